# revision 1
# baseline (speedup 1.0000x reference)
"""Trainium2 Bass kernel for nn_Attention — instruction-count-optimized v2."""
import numpy as np

HEADS = 4
HD = 32
DIM = 256
N = 4096
NQ = 2048
EPS = 1e-12
N_CORES = 8

_cache = {}
USE_T32_DMA = True
DIRECT_ROW_WRITE = False


def _build(reps: int = 1):
    import concourse.bass as bass
    import concourse.tile as tile
    from concourse import bacc, mybir
    from concourse.tile_rust import add_dep_helper

    F32 = mybir.dt.float32
    F32R = mybir.dt.float32r
    AF = mybir.ActivationFunctionType

    nc = bacc.Bacc("TRN2", target_bir_lowering=False, debug=False,
                   num_devices=N_CORES)

    x_in = nc.dram_tensor("x", [DIM, N], F32, kind="ExternalInput")
    xq_in = nc.dram_tensor("xq", [DIM, NQ], F32, kind="ExternalInput")
    wqt_in = nc.dram_tensor("wqt", [DIM, 128], F32, kind="ExternalInput")
    wkt_in = nc.dram_tensor("wkt", [DIM, 128], F32, kind="ExternalInput")
    wvt_in = nc.dram_tensor("wvt", [DIM, 128], F32, kind="ExternalInput")
    wot_in = nc.dram_tensor("wot", [128, DIM], F32, kind="ExternalInput")
    bo_in = nc.dram_tensor("bo", [DIM], F32, kind="ExternalInput")
    sel_in = nc.dram_tensor("sel", [128, 128], F32, kind="ExternalInput")
    out_dram = nc.dram_tensor("out", [DIM, NQ], F32, kind="ExternalOutput")
    scr_dram = nc.dram_tensor("scr", [1, N], F32, kind="ExternalOutput")

    with tile.TileContext(nc) as tc:
      with tc.tile_pool(name="consts", bufs=1) as consts, \
           tc.tile_pool(name="big", bufs=1) as big, \
           tc.tile_pool(name="ps_s4", bufs=1, space="PSUM") as ps_s4, \
           tc.tile_pool(name="ps_o", bufs=4, space="PSUM") as ps_o:

        for rep in range(reps):
            # ---------------- phase 0 ----------------
            with tc.tile_pool(name=f"xpool{rep}", bufs=1) as xpool, \
                 tc.tile_pool(name=f"ldp{rep}", bufs=1) as ldp, \
                 tc.tile_pool(name=f"sqp{rep}", bufs=2) as sqp, \
                 tc.tile_pool(name=f"work{rep}", bufs=1) as work:

                def load_f32r(name, shape, src_ap):
                    t_ld = ldp.tile(shape, F32, tag="wld", name=name + "_ld")
                    nc.sync.dma_start(out=t_ld, in_=src_ap)
                    t_r = consts.tile(shape, F32R, tag=name, name=name)
                    nc.vector.tensor_copy(t_r, t_ld)
                    return t_r

                wqt_r = load_f32r("wqt", [128, 2, 128],
                                  wqt_in.rearrange("(cc p) m -> p cc m", p=128))
                wkt_r = load_f32r("wkt", [128, 2, 128],
                                  wkt_in.rearrange("(cc p) m -> p cc m", p=128))
                wvt_r = load_f32r("wvt", [128, 2, 128],
                                  wvt_in.rearrange("(cc p) m -> p cc m", p=128))
                wot_r = load_f32r("wot", [128, 256], wot_in[:, :])
                sel_r = load_f32r("sel", [128, 128], sel_in[:, :])
                recs128 = big.tile([128, 512], F32, tag="recs")
                nc.vector.memset(recs128, 1.0)
                recf = big.tile([128, 512], F32, tag="recf")
                recsr = big.tile([128, 512], F32R, tag="recsr")

                bo_ld = ldp.tile([128, 2], F32, tag="wld", name="bo_ld")
                nc.sync.dma_start(out=bo_ld,
                                  in_=bo_in.rearrange("(cc p) -> p cc", p=128))
                bo_sb = consts.tile([128, 2], F32, tag="bo")
                nc.vector.tensor_copy(bo_sb, bo_ld)

                ones_128f = ldp.tile([128, 128], F32, tag="wld", name="ones128f")
                nc.vector.memset(ones_128f, 1.0)
                ones_128 = consts.tile([128, 128], F32R, tag="ones128")
                nc.vector.tensor_copy(ones_128, ones_128f)
                ones_c1 = ones_128[:, 0:1]
                eps_sb = consts.tile([128, 1], F32, tag="eps")
                nc.vector.memset(eps_sb, EPS)

                # x loads + f32r copies (keep F32 loads for squares)
                xr, x_f = [], []
                for cc in range(2):
                    t_ld = ldp.tile([128, N], F32, tag=f"xld{cc}", name=f"x_ld{cc}")
                    nc.sync.dma_start(out=t_ld, in_=x_in[128 * cc:128 * (cc + 1), :])
                    t_r = xpool.tile([128, N], F32R, tag=f"xr{cc}", name=f"xr{cc}")
                    nc.vector.tensor_copy(t_r, t_ld)
                    xr.append(t_r)
                    x_f.append(t_ld)

                # full-token inverse RMS -> T32 [128, 32] via DRAM roundtrip
                t32 = consts.tile([128, 32], F32, tag="t32")
                scr_writes = []
                for half in range(2):
                    srow = ps_s4.tile([1, 2048], F32, tag="s4", name=f"srow{half}")
                    for cc in range(2):
                        xsq = sqp.tile([128, 2048], F32R, tag="xsq",
                                       name=f"xsq{half}_{cc}")
                        nc.vector.tensor_mul(
                            xsq, x_f[cc][:, 2048 * half:2048 * (half + 1)],
                            x_f[cc][:, 2048 * half:2048 * (half + 1)])
                        for tb in range(4):
                            sl2 = slice(512 * tb, 512 * (tb + 1))
                            nc.tensor.matmul(srow[:, sl2], ones_c1, xsq[:, sl2],
                                             start=(cc == 0), stop=(cc == 1))
                    rms_h = work.tile([1, 2048], F32, tag="rms_h",
                                      name=f"rms_h{half}")
                    nc.scalar.activation(rms_h, srow, AF.Sqrt,
                                         scale=1.0 / DIM, bias=eps_sb[0:1, :])
                    inv_h = work.tile([1, 2048], F32, tag="inv_h",
                                      name=f"inv_h{half}")
                    scr_h = ps_s4.tile([1, 2048], F32, tag="s4",
                                       name=f"scr_h{half}")
                    nc.vector.reciprocal_approx_accurate(inv_h, rms_h, scr_h)
                    w_i = nc.sync.dma_start(
                        out=scr_dram[:, 2048 * half:2048 * (half + 1)], in_=inv_h)
                    scr_writes.append(w_i)
                r_i = nc.scalar.dma_start(
                    out=t32, in_=scr_dram.rearrange("1 (c p) -> p c", p=128))
                for w_i in scr_writes:
                    add_dep_helper(r_i.ins, w_i.ins, sync=True,
                                   reason="scr roundtrip write-before-read")

                # xq loads + squares + inverse RMS broadcast [128, NQ]
                xqr, xq_f = [], []
                for cc in range(2):
                    t_ld = ldp.tile([128, NQ], F32, tag=f"xld{cc}", name=f"xq_ld{cc}")
                    nc.sync.dma_start(out=t_ld, in_=xq_in[128 * cc:128 * (cc + 1), :])
                    t_r = xpool.tile([128, NQ], F32R, tag=f"xqr{cc}", name=f"xqr{cc}")
                    nc.vector.tensor_copy(t_r, t_ld)
                    xqr.append(t_r)
                    xq_f.append(t_ld)
                invq = xpool.tile([128, NQ], F32, tag="invq")
                rmsq = xpool.tile([128, NQ], F32, tag="rmsq")
                qss = ps_s4.tile([128, 2048], F32, tag="s4", name="qss")
                for cc in range(2):
                    xqsq = sqp.tile([128, 2048], F32R, tag="xsq", name=f"xqsq{cc}")
                    nc.vector.tensor_mul(xqsq, xq_f[cc], xq_f[cc])
                    for tb in range(4):
                        sl2 = slice(512 * tb, 512 * (tb + 1))
                        nc.tensor.matmul(qss[:, sl2], ones_128, xqsq[:, sl2],
                                         start=(cc == 0), stop=(cc == 1))
                nc.scalar.activation(rmsq, qss, AF.Sqrt, scale=1.0 / DIM,
                                     bias=eps_sb)
                qscr = ps_s4.tile([128, 2048], F32, tag="s4", name="qscr")
                nc.vector.reciprocal_approx_accurate(invq, rmsq, qscr)

                # K projection -> kr [128, N] f32r
                kr = big.tile([128, N], F32R, tag="kr")
                for half in range(2):
                    kps = ps_s4.tile([128, 2048], F32, tag="s4", name=f"kps{half}")
                    for tb in range(4):
                        sl2 = slice(512 * tb, 512 * (tb + 1))
                        gsl = slice(2048 * half + 512 * tb,
                                    2048 * half + 512 * (tb + 1))
                        for cc in range(2):
                            nc.tensor.matmul(kps[:, sl2], wkt_r[:, cc, :],
                                             xr[cc][:, gsl],
                                             start=(cc == 0), stop=(cc == 1))
                    nc.vector.tensor_copy(kr[:, 2048 * half:2048 * (half + 1)], kps)

                # Q projection (x invq) -> qr [128, NQ] f32r
                qr = big.tile([128, NQ], F32R, tag="qr")
                qps = ps_s4.tile([128, 2048], F32, tag="s4", name="qps")
                for tb in range(4):
                    sl2 = slice(512 * tb, 512 * (tb + 1))
                    for cc in range(2):
                        nc.tensor.matmul(qps[:, sl2], wqt_r[:, cc, :],
                                         xqr[cc][:, sl2],
                                         start=(cc == 0), stop=(cc == 1))
                nc.vector.tensor_mul(qr, qps, invq)

                # V^T with invrms(t) scaling and ones columns -> v4 [128, 32, 132]
                v4 = big.tile([128, 32, 132], F32R, tag="v4")
                ones_v = v4.rearrange("p t (h x) -> p t h x", x=33)[:, :, :, 32]
                nc.vector.tensor_copy(
                    ones_v, ones_128.rearrange("p (t h) -> p t h", h=4))
                for tb in range(32):
                    vps = ps_o.tile([128, 128], F32, tag="o", name=f"vps{tb}")
                    for cc in range(2):
                        nc.tensor.matmul(vps, xr[cc][:, 128 * tb:128 * (tb + 1)],
                                         wvt_r[:, cc, :],
                                         start=(cc == 0), stop=(cc == 1))
                    nc.vector.tensor_scalar_mul(
                        v4[:, tb, :].rearrange("p (h x) -> p h x", x=33)[:, :, 0:32],
                        vps.rearrange("p (h x) -> p h x", x=32),
                        t32[:, tb:tb + 1])

            # ---------------- attention main loop ----------------
            with tc.tile_pool(name=f"mainp{rep}", bufs=2) as mainp:
                epool = onpool = outpool = norm = mainp
                for ib in range(4):
                    isl = slice(512 * ib, 512 * (ib + 1))
                    o_h = [ps_o.tile([33, 512], F32, tag="o",
                                     name=f"o_h{rep}_{ib}_{h}")
                           for h in range(4)]
                    for jb in range(32):
                        s4 = ps_s4.tile([128, 2048], F32, tag="s4",
                                        name=f"s4_{rep}_{ib}_{jb}")
                        for h in range(4):
                            nc.tensor.matmul(
                                s4[:, 512 * h:512 * (h + 1)],
                                kr[32 * h:32 * h + 32,
                                   128 * jb:128 * (jb + 1)],
                                qr[32 * h:32 * h + 32, isl],
                                start=True, stop=True,
                                tile_position=(32 * h, 0))
                        e4 = epool.tile([128, 2048], F32R, tag="e4",
                                        name=f"e4_{rep}_{ib}_{jb}")
                        nc.scalar.activation(e4, s4, AF.Exp,
                                             scale=t32[:, jb:jb + 1])
                        for h in range(4):
                            nc.tensor.matmul(
                                o_h[h], v4[:, jb, 33 * h:33 * (h + 1)],
                                e4[:, 512 * h:512 * (h + 1)],
                                start=(jb == 0), stop=(jb == 31))

                    # normalization
                    for h in range(4):
                        nc.vector.tensor_copy(recs128[32 * h:32 * h + 1, :],
                                              o_h[h][32:33, :])
                    rsc = ps_s4.tile([128, 512], F32, tag="s4",
                                     name=f"rsc{rep}_{ib}")
                    nc.vector.reciprocal_approx_accurate(recf, recs128, rsc)
                    with nc.allow_low_precision(reason="f32r recip bcast"):
                        nc.vector.tensor_copy(recsr, recf)
                    b_ps = ps_s4.tile([128, 512], F32, tag="s4",
                                      name=f"bps{rep}_{ib}")
                    nc.tensor.matmul(b_ps, sel_r, recsr, start=True, stop=True)
                    b_sb = norm.tile([128, 512], F32, tag="b_sb",
                                     name=f"bsb{rep}_{ib}")
                    nc.vector.tensor_copy(b_sb, b_ps)
                    on_t = onpool.tile([128, 512], F32R, tag="on",
                                       name=f"on{rep}_{ib}")
                    for h in range(4):
                        nc.vector.tensor_mul(on_t[32 * h:32 * h + 32, :],
                                             o_h[h][0:32, :],
                                             b_sb[32 * h:32 * h + 32, :])

                    # output projection + bias; DMA once per 1024-col pair
                    if ib % 2 == 0:
                        osb_pair = [outpool.tile([128, 1024], F32, tag=f"osb{oc}",
                                                 name=f"osb{rep}_{ib}_{oc}")
                                    for oc in range(2)]
                    for oc in range(2):
                        pps = ps_o.tile([128, 512], F32, tag="o",
                                        name=f"pps{rep}_{ib}_{oc}")
                        nc.tensor.matmul(pps,
                                         wot_r[:, 128 * oc:128 * (oc + 1)],
                                         on_t, start=True, stop=True)
                        nc.vector.tensor_scalar_add(
                            osb_pair[oc][:, 512 * (ib % 2):512 * (ib % 2 + 1)],
                            pps, bo_sb[:, oc:oc + 1])
                        if ib % 2 == 1:
                            nc.sync.dma_start(
                                out=out_dram[128 * oc:128 * (oc + 1),
                                             1024 * (ib // 2):1024 * (ib // 2 + 1)],
                                in_=osb_pair[oc])


    nc.compile()
    return nc


def _get_nc(reps: int = 1):
    if reps not in _cache:
        _cache[reps] = _build(reps)
    return _cache[reps]


def _prep_inputs(x, g, w_qkv, w_out, b_out):
    x = np.asarray(x, np.float32)
    g = np.asarray(g, np.float32)
    w_qkv = np.asarray(w_qkv, np.float32)
    w_out = np.asarray(w_out, np.float32)
    b_out = np.asarray(b_out, np.float32)

    wq = (w_qkv[0:128] * g[None, :]) * (HD ** -0.5)
    wk = w_qkv[128:256] * g[None, :]
    wv = w_qkv[256:384] * g[None, :]
    sel = np.zeros((128, 128), np.float32)
    for h in range(4):
        sel[32 * h, 32 * h:32 * h + 32] = 1.0

    b, c, hh, ww = x.shape
    xf = x.reshape(b, c, hh * ww)
    in_maps = []
    for core in range(N_CORES):
        beta, tau = core // 2, core % 2
        in_maps.append({
            "x": np.ascontiguousarray(xf[beta]),
            "xq": np.ascontiguousarray(xf[beta][:, NQ * tau:NQ * (tau + 1)]),
            "wqt": np.ascontiguousarray(wq.T),
            "wkt": np.ascontiguousarray(wk.T),
            "wvt": np.ascontiguousarray(wv.T),
            "wot": np.ascontiguousarray(w_out.T),
            "bo": b_out, "sel": sel,
        })
    return in_maps


def _run(in_maps, reps: int = 1):
    from concourse.bass_utils import run_bass_kernel_spmd
    nc = _get_nc(reps)
    return run_bass_kernel_spmd(nc, in_maps, list(range(N_CORES))).results


def kernel(x, g, w_qkv, w_out, b_out):
    x = np.asarray(x, np.float32)
    b, c, hh, ww = x.shape
    in_maps = _prep_inputs(x, g, w_qkv, w_out, b_out)
    results = _run(in_maps, reps=1)
    out = np.empty((b, DIM, hh * ww), np.float32)
    for core in range(N_CORES):
        beta, tau = core // 2, core % 2
        out[beta][:, NQ * tau:NQ * (tau + 1)] = results[core]["out"]
    return out.reshape(b, DIM, hh, ww)

